# revision 3
# baseline (speedup 1.0000x reference)
"""Trainium2 Bass kernel for 3x3 VALID conv (NCHW, stride 1) via banded-Toeplitz GEMM.

Full input (64, 8, 256, 256) f32 + filter (8, 8, 3, 3) -> output (64, 8, 254, 254).
Data-parallel over batch: 8 images per NeuronCore, 8 cores.

Single-row partition layout: SBUF partition (c, h) holds ONE input row for all
8 local images ([n, w] = 4 KB contiguous -> 4 KB DMA descriptors, SWDGE across
all 16 SDMA engines).  Contraction folds ALL THREE filter rows r into the
partition dim via a banded Toeplitz weight:

  W_s[(c,h), (m,q)] = f[m, c, h-q, s]   for 0 <= h-q <= 2

Per block of Q=14 output rows (hbn=16 input rows, kk=128, mm=112) and per
image pair, just 3 matmuls (one per filter column s, N = 2*254 = 508)
accumulate the full conv into PSUM.  That is 3 MMs / 14 rows vs the 12 MMs /
30 rows of the row-pair scheme -> ~2x less PE streaming.

Output is cast f32->bf16 on-chip (DVE/ACT split) and stored as [m, i, n, j]
so each store descriptor is the 4 KB [n, j] slab of one output row; the host
transposes back to NCHW and upcasts.  Total HBM traffic/core ~17.8 MB.
"""

import numpy as np

import concourse.bacc as bacc
import concourse.bass as bass
import concourse.mybir as mybir
import concourse.tile as tile
from concourse import bass_utils

F32 = mybir.dt.float32
BF16 = mybir.dt.bfloat16

N_CORES = 8
N_LOC = 8  # images per core
C, H, W = 8, 256, 256
M, R, S = 8, 3, 3
HO, WO = H - R + 1, W - S + 1  # 254, 254
Q = 14  # output rows per full block
HB = Q + R - 1  # 16 input rows per full block
NBLK = 18  # full blocks -> output rows 0..251
QT = 2  # tail outputs (252, 253)
HBT = QT + R - 1  # 4 tail input rows (252..255)

_CACHE = {}


def _band_weights(f, q_cnt):
    """w[(c,h), s, (m,q)] = f[m, c, h-q, s] for 0 <= h-q < R."""
    hbn = q_cnt + R - 1
    out = np.zeros((C * hbn, S, M * q_cnt), np.float32)
    for c in range(C):
        for m in range(M):
            for q in range(q_cnt):
                for r in range(R):
                    for s in range(S):
                        out[c * hbn + q + r, s, m * q_cnt + q] = f[m, c, r, s]
    return out


def _build_program():
    nc = bacc.Bacc("TRN2", target_bir_lowering=False, debug=False)
    x = nc.dram_tensor("x", [C, H, N_LOC, W], BF16, kind="ExternalInput").ap()
    w = nc.dram_tensor("w", [C * HB, S, M * Q], BF16, kind="ExternalInput").ap()
    wt = nc.dram_tensor("wt", [C * HBT, S, M * QT], BF16, kind="ExternalInput").ap()
    y = nc.dram_tensor("y", [M, HO, N_LOC, WO], BF16, kind="ExternalOutput").ap()

    with tile.TileContext(nc) as tc:
        with (
            tc.tile_pool(name="wpool", bufs=1) as wpool,
            tc.tile_pool(name="xpool", bufs=3) as xpool,
            tc.tile_pool(name="opool", bufs=2) as opool,
            tc.tile_pool(name="psum", bufs=6, space=bass.MemorySpace.PSUM) as pspool,
        ):
            wtile = wpool.tile([C * HB, S, M * Q], BF16, tag="w")
            nc.gpsimd.dma_start(wtile[:], w[:])
            wttile = wpool.tile([C * HBT, S, M * QT], BF16, tag="wt")
            nc.gpsimd.dma_start(wttile[:], wt[:])

            for b in range(NBLK + 1):
                if b < NBLK:
                    i0, q_cnt, wsel = Q * b, Q, wtile
                else:
                    i0, q_cnt, wsel = Q * NBLK, QT, wttile
                hbn = q_cnt + R - 1
                kk = C * hbn
                mm = M * q_cnt

                # [(c,h), n, w]; one 4 KB descriptor per partition
                xt = xpool.tile([kk, N_LOC, W], BF16, tag="xt")
                nc.gpsimd.dma_start(xt[:], x[:, i0 : i0 + hbn, :, :])

                ot = opool.tile([mm, N_LOC, WO], BF16, tag="ot")
                for p in range(N_LOC // 2):
                    ps = pspool.tile([mm, 2, WO], F32, tag="ps")
                    for s in range(S):
                        nc.tensor.matmul(
                            ps[:],
                            wsel[:, s, :],
                            xt[:, 2 * p : 2 * p + 2, s : s + WO],
                            start=(s == 0),
                            stop=(s == S - 1),
                        )
                    if p % 2 == 0:
                        nc.vector.tensor_copy(ot[:, 2 * p : 2 * p + 2, :], ps[:])
                    else:
                        nc.scalar.copy(ot[:, 2 * p : 2 * p + 2, :], ps[:])

                nc.gpsimd.dma_start(y[:, i0 : i0 + q_cnt, :, :], ot[:])
    nc.compile()
    return nc


def _get_program():
    if "nc" not in _CACHE:
        _CACHE["nc"] = _build_program()
    return _CACHE["nc"]


def _to_bf16(a):
    import ml_dtypes

    return np.ascontiguousarray(np.asarray(a, np.float32)).astype(ml_dtypes.bfloat16)


def _make_inputs(x_full, f):
    """Per-core input dicts. x_full: (64, 8, 256, 256) f32/bf16-able."""
    w_full = _to_bf16(_band_weights(f, Q))
    w_tail = _to_bf16(_band_weights(f, QT))
    maps = []
    for core in range(N_CORES):
        x8 = np.asarray(x_full[core * N_LOC : (core + 1) * N_LOC], np.float32)
        # [n, c, h, w] -> [c, h, n, w]
        xdev = _to_bf16(x8.transpose(1, 2, 0, 3))
        maps.append({"x": xdev, "w": w_full, "wt": w_tail})
    return maps


def kernel(_input, _filter):
    f = np.asarray(_filter, dtype=np.float32)
    nc = _get_program()
    in_maps = _make_inputs(_input, f)
    res = bass_utils.run_bass_kernel_spmd(nc, in_maps, core_ids=list(range(N_CORES)))
    # y is [m, i, n, j] bf16 per core -> [n, m, i, j] f32
    outs = [
        np.asarray(r["y"]).transpose(2, 0, 1, 3).astype(np.float32)
        for r in res.results
    ]
    return np.concatenate(outs, axis=0)


# revision 8
# speedup vs baseline: 1.1819x; 1.1819x over previous
"""Trainium2 Bass kernel for 3x3 VALID conv (NCHW, stride 1) via banded-Toeplitz GEMM.

Full input (64, 8, 256, 256) f32 + filter (8, 8, 3, 3) -> output (64, 8, 254, 254).
Data-parallel over batch: 8 images per NeuronCore, 8 cores.

PE: SBUF partition (c, h) holds ONE input row for all 8 local images.  The
contraction folds ALL THREE filter rows r into the partition dim via a banded
Toeplitz weight W_s[(c,h), (q,m)] = f[m, c, h-q, s] (0 <= h-q <= 2), so each
block of Q=14 output rows x 2 images needs just 3 matmuls (one per filter
column s, N=508) per image pair: 228 MMs total vs 432 in the row-pair scheme.

DMA: the 16 SDMA engines are statically partition-swizzled; HWDGE (sync/
scalar) only reaches E64-71 while SWDGE (gpsimd) reaches all 16, with
engine(p) in E72-79 iff (p mod 32) >= 16.  Every load/store is therefore
split in two: the (p mod 32) < 16 half goes HWDGE, the other half SWDGE, so
all 16 engines stay busy.  Input layout [c, h, n, w] and output layout
[i, m, n, j] (both host-side transposes) make every descriptor a contiguous
4 KB slab.  Loads are issued 2 blocks ahead so store semaphore waits never
stall load descriptor generation.  Output is cast f32->bf16 on-chip.
"""

import numpy as np

import concourse.bacc as bacc
import concourse.bass as bass
import concourse.mybir as mybir
import concourse.tile as tile
from concourse import bass_utils

F32 = mybir.dt.float32
BF16 = mybir.dt.bfloat16

N_CORES = 8
N_LOC = 8  # images per core
C, H, W = 8, 256, 256
M, R, S = 8, 3, 3
HO, WO = H - R + 1, W - S + 1  # 254, 254
Q = 14  # output rows per full block
HB = Q + R - 1  # 16 input rows per full block
NBLK = 18  # full blocks -> output rows 0..251
QT = 2  # tail outputs (252, 253)
HBT = QT + R - 1  # 4 tail input rows (252..255)
PF = 2  # load prefetch distance (blocks)
# blocks whose SWDGE half is diverted to HWDGE to balance E64-71 vs E72-79
HW_B_BLOCKS = frozenset()

_CACHE = {}


def _band_weights(f, q_cnt):
    """w[(c,h), s, (q,m)] = f[m, c, h-q, s] for 0 <= h-q < R (cols q-major)."""
    hbn = q_cnt + R - 1
    out = np.zeros((C * hbn, S, q_cnt * M), np.float32)
    for c in range(C):
        for m in range(M):
            for q in range(q_cnt):
                for r in range(R):
                    for s in range(S):
                        out[c * hbn + q + r, s, q * M + m] = f[m, c, r, s]
    return out


def _issue_load(nc, x, xpool, b):
    """Issue the (HWDGE+SWDGE) split input load for block b; returns tile."""
    if b < NBLK:
        i0, hbn = Q * b, HB
    else:
        i0, hbn = Q * NBLK, HBT
    kk = C * hbn
    xt = xpool.tile([kk, N_LOC, W], BF16, tag="xt")
    hw_eng = nc.sync if b % 2 == 0 else nc.scalar
    sw_eng = hw_eng if b in HW_B_BLOCKS else nc.gpsimd
    if b < NBLK:
        # partition p = c*16 + h; (p%32)<16 <=> c even -> E64-71 (HWDGE),
        # c odd -> E72-79 (SWDGE).  One chunk instruction per channel.
        for cc in range(C // 2):
            hw_eng.dma_start(
                xt[cc * 32 : cc * 32 + HB],
                x[2 * cc : 2 * cc + 1, i0 : i0 + hbn, :, :],
            )
            sw_eng.dma_start(
                xt[cc * 32 + HB : cc * 32 + 2 * HB],
                x[2 * cc + 1 : 2 * cc + 2, i0 : i0 + hbn, :, :],
            )
    else:
        # tail: p = c*4 + h; (p%32)<16 <=> c < 4: plain halves
        hw_eng.dma_start(xt[: kk // 2], x[: C // 2, i0 : i0 + hbn, :, :])
        sw_eng.dma_start(xt[kk // 2 :], x[C // 2 :, i0 : i0 + hbn, :, :])
    return xt


def _build_program():
    nc = bacc.Bacc("TRN2", target_bir_lowering=False, debug=False)
    x = nc.dram_tensor("x", [C, H, N_LOC, W], BF16, kind="ExternalInput").ap()
    w = nc.dram_tensor("w", [C * HB, S, Q * M], BF16, kind="ExternalInput").ap()
    wt = nc.dram_tensor("wt", [C * HBT, S, QT * M], BF16, kind="ExternalInput").ap()
    y = nc.dram_tensor("y", [HO, M, N_LOC, WO], BF16, kind="ExternalOutput").ap()

    with tile.TileContext(nc) as tc:
        with (
            tc.tile_pool(name="wpool", bufs=1) as wpool,
            tc.tile_pool(name="xpool", bufs=PF + 2) as xpool,
            tc.tile_pool(name="opool", bufs=3) as opool,
            tc.tile_pool(name="psum", bufs=8, space=bass.MemorySpace.PSUM) as pspool,
        ):
            wtile = wpool.tile([C * HB, S, Q * M], BF16, tag="w")
            nc.sync.dma_start(wtile[:], w[:])
            wttile = wpool.tile([C * HBT, S, QT * M], BF16, tag="wt")
            nc.sync.dma_start(wttile[:], wt[:])

            xts = {b: _issue_load(nc, x, xpool, b) for b in range(min(PF, NBLK + 1))}

            for b in range(NBLK + 1):
                if b + PF <= NBLK:
                    xts[b + PF] = _issue_load(nc, x, xpool, b + PF)
                xt = xts.pop(b)
                if b < NBLK:
                    i0, q_cnt, wsel = Q * b, Q, wtile
                else:
                    i0, q_cnt, wsel = Q * NBLK, QT, wttile
                mm = q_cnt * M

                # 128 partitions so the (qq t m) store rearrange divides evenly
                ot = opool.tile([128 if q_cnt == Q else mm, N_LOC, WO], BF16, tag="ot")
                for p in range(N_LOC // 2):
                    ps = pspool.tile([mm, 2, WO], F32, tag="ps")
                    for s in range(S):
                        nc.tensor.matmul(
                            ps[:],
                            wsel[:, s, :],
                            xt[:, 2 * p : 2 * p + 2, s : s + WO],
                            start=(s == 0),
                            stop=(s == S - 1),
                        )
                    if p == 2:
                        nc.scalar.copy(ot[:mm, 2 * p : 2 * p + 2, :], ps[:])
                    else:
                        nc.vector.tensor_copy(ot[:mm, 2 * p : 2 * p + 2, :], ps[:])

                hw_eng = nc.scalar if b % 2 == 0 else nc.sync
                sw_eng = hw_eng if b in HW_B_BLOCKS else nc.gpsimd
                if b < NBLK:
                    # cols p = q*8+m; (p%32)<16 <=> q%4 in {0,1} -> HWDGE.
                    for qq in range(4):
                        hw_eng.dma_start(
                            y[i0 + 4 * qq : i0 + 4 * qq + 2, :, :, :],
                            ot[qq * 32 : qq * 32 + 16],
                        )
                        if qq < 3:
                            sw_eng.dma_start(
                                y[i0 + 4 * qq + 2 : i0 + 4 * qq + 4, :, :, :],
                                ot[qq * 32 + 16 : qq * 32 + 32],
                            )
                else:
                    # tail: cols 0..15 all in HWDGE half
                    hw_eng.dma_start(y[i0 : i0 + QT, :, :, :], ot[:])
    nc.compile()
    return nc


def _get_program():
    if "nc" not in _CACHE:
        _CACHE["nc"] = _build_program()
    return _CACHE["nc"]


def _to_bf16(a):
    import ml_dtypes

    return np.ascontiguousarray(np.asarray(a, np.float32)).astype(ml_dtypes.bfloat16)


def _make_inputs(x_full, f):
    """Per-core input dicts. x_full: (64, 8, 256, 256)."""
    w_full = _to_bf16(_band_weights(f, Q))
    w_tail = _to_bf16(_band_weights(f, QT))
    maps = []
    for core in range(N_CORES):
        x8 = np.asarray(x_full[core * N_LOC : (core + 1) * N_LOC], np.float32)
        # [n, c, h, w] -> [c, h, n, w]
        xdev = _to_bf16(x8.transpose(1, 2, 0, 3))
        maps.append({"x": xdev, "w": w_full, "wt": w_tail})
    return maps


def kernel(_input, _filter):
    f = np.asarray(_filter, dtype=np.float32)
    nc = _get_program()
    in_maps = _make_inputs(_input, f)
    res = bass_utils.run_bass_kernel_spmd(nc, in_maps, core_ids=list(range(N_CORES)))
    # y is [i, m, n, j] bf16 per core -> [n, m, i, j] f32
    outs = [
        np.asarray(r["y"]).transpose(2, 1, 0, 3).astype(np.float32)
        for r in res.results
    ]
    return np.concatenate(outs, axis=0)


# revision 9
# speedup vs baseline: 1.5425x; 1.3050x over previous
"""Trainium2 Bass kernel for 3x3 VALID conv (NCHW, stride 1) via banded-Toeplitz GEMM.

Full input (64, 8, 256, 256) f32 + filter (8, 8, 3, 3) -> output (64, 8, 254, 254).

Sharding: 2-way over batch x 4-way over output rows (core = ns*4 + hs).
Each core handles 32 images x 64 output rows (row shard 3 starts at 190 and
recomputes rows 190-191 so every shard runs the identical program).

PE: SBUF partition (h*8+c) holds ONE input row for the 32 local images.  The
contraction folds ALL THREE filter rows r into the partition dim via a banded
Toeplitz weight W_s[(h,c), (q,m)] = f[m, c, h-q, s], so a block of Q=14
output rows x 2 images needs just 3 matmuls (one per filter column s, N=508).
A block covers 16 image pairs -> 10us of back-to-back PE work, enough to keep
the HAM clock gate at 2.4 GHz.

DMA: with 8 cores running, per-core HBM sustains only ~250-350 GB/s, so
bytes are everything: bf16 input (8.6 MB) + bf16 output (8.3 MB) per core,
each row loaded from HBM exactly once (block-boundary rows are copied
SBUF->SBUF between consecutive block tiles, 1 instruction per boundary).
Input [h, c, n, w] and output [i, m, n, j] layouts (host-side transposes)
give 16 KB contiguous descriptors.  Loads are issued 2 blocks ahead on the
single SWDGE queue so store semaphore waits never stall load generation.
Output is cast f32->bf16 on-chip (vector/scalar alternating).
"""

import numpy as np

import concourse.bacc as bacc
import concourse.bass as bass
import concourse.mybir as mybir
import concourse.tile as tile
from concourse import bass_utils

F32 = mybir.dt.float32
BF16 = mybir.dt.bfloat16

N_CORES = 8
N_LOC = 32  # images per core (2-way batch shard)
NROW = 64  # output rows per core (4-way row shard)
C, H, W = 8, 256, 256
M, R, S = 8, 3, 3
HO, WO = H - R + 1, W - S + 1  # 254, 254
Q = 14  # output rows per full block
HB = Q + R - 1  # 16 input rows per full block
QT = 8  # tail block outputs (56..63)
HBT = QT + R - 1  # 10
BLOCKS = [(0, Q), (14, Q), (28, Q), (42, Q), (56, QT)]
NB = len(BLOCKS)
PF = 2  # load prefetch distance (blocks)
HROWS = NROW + R - 1  # 66 input rows per core

_CACHE = {}


def _row_start(hs):
    return 64 * hs if hs < 3 else 190


def _band_weights(f, q_cnt):
    """w[(h,c), s, (q,m)] = f[m, c, h-q, s] for 0 <= h-q < R."""
    hbn = q_cnt + R - 1
    out = np.zeros((hbn * C, S, q_cnt * M), np.float32)
    for c in range(C):
        for m in range(M):
            for q in range(q_cnt):
                for r in range(R):
                    for s in range(S):
                        out[(q + r) * C + c, s, q * M + m] = f[m, c, r, s]
    return out


def _build_program():
    nc = bacc.Bacc("TRN2", target_bir_lowering=False, debug=False)
    x = nc.dram_tensor("x", [HROWS, C, N_LOC, W], BF16, kind="ExternalInput").ap()
    w = nc.dram_tensor("w", [HB * C, S, Q * M], BF16, kind="ExternalInput").ap()
    wt = nc.dram_tensor("wt", [HBT * C, S, QT * M], BF16, kind="ExternalInput").ap()
    y = nc.dram_tensor("y", [NROW, M, N_LOC, WO], BF16, kind="ExternalOutput").ap()

    with tile.TileContext(nc) as tc:
        with (
            tc.tile_pool(name="wpool", bufs=1) as wpool,
            tc.tile_pool(name="xpool", bufs=PF + 2) as xpool,
            tc.tile_pool(name="opool", bufs=2) as opool,
            tc.tile_pool(name="psum", bufs=8, space=bass.MemorySpace.PSUM) as pspool,
        ):
            wtile = wpool.tile([HB * C, S, Q * M], BF16, tag="w")
            nc.sync.dma_start(wtile[:], w[:])
            wttile = wpool.tile([HBT * C, S, QT * M], BF16, tag="wt")
            nc.sync.dma_start(wttile[:], wt[:])

            xts = {}

            def issue_load(b):
                i0, q_cnt = BLOCKS[b]
                hbn = q_cnt + R - 1
                xt = xpool.tile([hbn * C, N_LOC, W], BF16, tag="xt")
                if b == 0:
                    nc.gpsimd.dma_start(xt[:], x[i0 : i0 + hbn])
                else:
                    # rows i0, i0+1 come from the previous tile (SBUF->SBUF);
                    # only rows i0+2 .. i0+hbn-1 are fetched from HBM.
                    nc.gpsimd.dma_start(
                        xt[: 2 * C], xts[b - 1][(HB - 2) * C : HB * C]
                    )
                    nc.gpsimd.dma_start(xt[2 * C :], x[i0 + 2 : i0 + hbn])
                xts[b] = xt

            for b in range(min(PF + 1, NB)):
                issue_load(b)

            for b in range(NB):
                if b + PF + 1 < NB:
                    issue_load(b + PF + 1)
                xt = xts[b]
                i0, q_cnt = BLOCKS[b]
                wsel = wtile if q_cnt == Q else wttile
                mm = q_cnt * M

                ot = opool.tile([mm, N_LOC, WO], BF16, tag="ot")
                for p in range(N_LOC // 2):
                    ps = pspool.tile([mm, 2, WO], F32, tag="ps")
                    for s in range(S):
                        nc.tensor.matmul(
                            ps[:],
                            wsel[:, s, :],
                            xt[:, 2 * p : 2 * p + 2, s : s + WO],
                            start=(s == 0),
                            stop=(s == S - 1),
                        )
                    if p % 2 == 0:
                        nc.vector.tensor_copy(ot[:, 2 * p : 2 * p + 2, :], ps[:])
                    else:
                        nc.scalar.copy(ot[:, 2 * p : 2 * p + 2, :], ps[:])

                nc.gpsimd.dma_start(y[i0 : i0 + q_cnt], ot[:])
    nc.compile()
    return nc


def _get_program():
    if "nc" not in _CACHE:
        _CACHE["nc"] = _build_program()
    return _CACHE["nc"]


def _to_bf16(a):
    import ml_dtypes

    return np.ascontiguousarray(np.asarray(a, np.float32)).astype(ml_dtypes.bfloat16)


def _make_inputs(x_full, f):
    """Per-core input dicts. x_full: (64, 8, 256, 256)."""
    w_full = _to_bf16(_band_weights(np.asarray(f, np.float32), Q))
    w_tail = _to_bf16(_band_weights(np.asarray(f, np.float32), QT))
    maps = []
    for core in range(N_CORES):
        ns, hs = divmod(core, 4)
        g0 = _row_start(hs)
        xs = np.asarray(
            x_full[32 * ns : 32 * ns + 32, :, g0 : g0 + HROWS, :], np.float32
        )
        # [n, c, h, w] -> [h, c, n, w]
        xdev = _to_bf16(xs.transpose(2, 1, 0, 3))
        maps.append({"x": xdev, "w": w_full, "wt": w_tail})
    return maps


def _assemble(results):
    out = np.empty((64, 8, HO, WO), np.float32)
    for core, r in enumerate(results):
        ns, hs = divmod(core, 4)
        g0 = _row_start(hs)
        # y: [i, m, n, j] -> [n, m, i, j]
        yt = np.asarray(r["y"]).transpose(2, 1, 0, 3).astype(np.float32)
        lo = 0 if hs < 3 else 2
        out[32 * ns : 32 * ns + 32, :, g0 + lo : g0 + NROW, :] = yt[:, :, lo:, :]
    return out


def kernel(_input, _filter):
    nc = _get_program()
    in_maps = _make_inputs(_input, _filter)
    res = bass_utils.run_bass_kernel_spmd(nc, in_maps, core_ids=list(range(N_CORES)))
    return _assemble(res.results)
